# revision 14
# baseline (speedup 1.0000x reference)
"""MinusAttention kernel for Trainium2 (8 NeuronCores, Bass/Tile).

Math: score[i,j] = (w.q_i - w.k_j + b) / sqrt(E) with causal mask.
Within a softmax row i the w.q_i and b terms cancel, so

    weights[i,j] = g_j / sum_{j'<=i} g_j',   g_j = exp(-w.k_j / sqrt(E))
    out[i,:]     = (sum_{j<=i} g_j V[j,:]) / (sum_{j<=i} g_j)

i.e. a causal cumulative weighted average of V -- O(S*E) per (b,h) --
and the output does not depend on queries at all.

v2: fp16 end-to-end (PE at 1 cycle/col vs 4 for fp32; half the HBM
traffic; DVE 2x/4x packed-16bit modes), e-major/k-last SBUF layout so
the two big DVE broadcasts (g along e, r along e) hit the fast packed
modes, PSUM chunked by e-halves ([128, 16blk x 32e] f32 = one 2KB
bank), consts DMA'd from host instead of generated on-device.

Per pair (b,h), s = 128*k + p (p partition, k block 0..15):
  kt  [128,16,64] f16  (host-prescaled by -w/sqrt(E))
  vg  [128,65,16] f16  (e-major, k-last; row e=64 is ones)
  sk  = reduce_add_e(kt)          DVE  -> [128,16] f32
  g   = exp(sk)                   ACT  -> [128,1,16] f16
  wg  = vg * g_bcast              DVE  (4x packed mode)
  ps[h] = triT @ wg[:,eh,:]^T(k-major view)   PE f16, h=0,1
  psD   = triT @ wg[:,64,:]                   PE (denominator column)
  c32 = copy(ps rows 96:128)      ACT+DVE (PSUM reads: 32-aligned base)
  bsT = scatter row 31            1 tiny DMA (block sums -> partitions)
  rm[h] = maskKE * bsT_bcast      DVE (carry terms, k' < k)
  ps[h] += ones16 @ rm[h]         PE (adds inter-block carries)
  r   = 1/psD_final               DVE
  cw  = copy(ps, transpose->e-major) ACT+Pool drains
  ot  = cw * r_bcast              DVE (4x packed mode)
  out DMA (f16; host upcasts)
"""

import numpy as np

B, L, S, H, E = 4, 2048, 2048, 8, 64
NCORES = 8
PAIRS = (B * H) // NCORES  # 4 (b,h) pairs per core
NBLK = S // 128  # 16
EH = 32  # e-half; PSUM chunk [128, NBLK*EH] f32 = 2KB = one bank
GROUP = 2
SCALE = np.float32(1.0 / np.sqrt(np.float32(E)))

TRACE = False
LAST_RESULTS = None

_compiled = None


def _build():
    from concourse import bacc
    import concourse.mybir as mybir
    import concourse.tile as tile

    f16 = mybir.dt.float16
    f32 = mybir.dt.float32
    nc = bacc.Bacc("TRN2", target_bir_lowering=False, debug=False)

    ktw = nc.dram_tensor("ktw", [PAIRS, 128, NBLK, E], f16, kind="ExternalInput")
    vg = nc.dram_tensor("vg", [PAIRS, 128, E + 1, NBLK], f16, kind="ExternalInput")
    tri_c = nc.dram_tensor("tri_c", [128, 128], f16, kind="ExternalInput")
    ones_c = nc.dram_tensor("ones_c", [16, 128], f16, kind="ExternalInput")
    # maskKED[k', k, e] = 1 iff k' < k for e in 0:65 (both e-halves + D col)
    mke_c = nc.dram_tensor("mke_c", [16, NBLK, 2 * EH + 1], f16, kind="ExternalInput")
    out = nc.dram_tensor("out", [PAIRS, 128, E, NBLK], f16, kind="ExternalOutput")

    with tile.TileContext(nc) as tc:
        with (
            nc.allow_low_precision(reason="fp16 kernel; harness gate is 2e-2"),
            tc.tile_pool(name="const", bufs=1) as cpool,
            tc.tile_pool(name="ktp", bufs=GROUP + 1) as ktp,
            tc.tile_pool(name="vgp", bufs=GROUP + 1) as vgp,
            tc.tile_pool(name="skp", bufs=2 * GROUP) as skp,
            tc.tile_pool(name="gp", bufs=2 * GROUP) as gp,
            tc.tile_pool(name="wgp", bufs=GROUP + 1) as wgp,
            tc.tile_pool(name="c32p", bufs=2 * GROUP) as c32p,
            tc.tile_pool(name="bsp", bufs=2 * GROUP) as bsp,
            tc.tile_pool(name="rmp", bufs=3 * GROUP) as rmp,
            tc.tile_pool(name="rp", bufs=2 * GROUP) as rp,
            tc.tile_pool(name="cwp", bufs=GROUP + 1) as cwp,
            tc.tile_pool(name="otp", bufs=GROUP + 1) as otp,
            tc.tile_pool(name="ps", bufs=6, space="PSUM") as psp,
            tc.tile_pool(name="psd", bufs=2, space="PSUM") as psdp,
        ):
            tri = cpool.tile([128, 128], f16)
            nc.sync.dma_start(out=tri[:], in_=tri_c[:])
            ones16 = cpool.tile([16, 128], f16)
            nc.sync.dma_start(out=ones16[:], in_=ones_c[:])
            maskKED = cpool.tile([16, NBLK, 2 * EH + 1], f16)
            nc.sync.dma_start(out=maskKED[:], in_=mke_c[:])

            for grp in range(PAIRS // GROUP):
                pairs = list(range(grp * GROUP, (grp + 1) * GROUP))

                kts, vgts = {}, {}
                for p in pairs:
                    kt = ktp.tile([128, NBLK, E], f16, tag="kt")
                    vgt = vgp.tile([128, E + 1, NBLK], f16, tag="vg")
                    nc.sync.dma_start(out=kt[:], in_=ktw[p])
                    nc.gpsimd.dma_start(out=vgt[:], in_=vg[p])
                    kts[p], vgts[p] = kt, vgt

                wgs = {}
                for p in pairs:
                    sk = skp.tile([128, NBLK], f16, tag="sk")
                    nc.vector.tensor_reduce(
                        sk[:], kts[p][:], mybir.AxisListType.X, mybir.AluOpType.add
                    )
                    g = gp.tile([128, 1, NBLK], f16, tag="g")
                    nc.scalar.activation(
                        g[:].rearrange("p o k -> p (o k)"), sk[:],
                        mybir.ActivationFunctionType.Exp,
                    )
                    wg = wgp.tile([128, E + 1, NBLK], f16, tag="wg")
                    nc.vector.tensor_tensor(
                        out=wg[:], in0=vgts[p][:],
                        in1=g[:].broadcast_to([128, E + 1, NBLK]),
                        op=mybir.AluOpType.mult,
                    )
                    wgs[p] = wg

                pss = {}
                for p in pairs:
                    for h in range(2):
                        ps = psp.tile([128, NBLK, EH], f32, tag="ps")
                        nc.tensor.matmul(
                            ps[:], lhsT=tri[:],
                            rhs=wgs[p][:, h * EH:(h + 1) * EH, :].rearrange(
                                "p e k -> p k e"),
                            start=True, stop=False, skip_group_check=True,
                        )
                        pss[(p, h)] = ps
                    psd = psdp.tile([128, NBLK], f32, tag="psd")
                    nc.tensor.matmul(
                        psd[:], lhsT=tri[:],
                        rhs=wgs[p][:, E:E + 1, :].rearrange("p o k -> p (o k)"),
                        start=True, stop=False, skip_group_check=True,
                    )
                    pss[(p, 2)] = psd

                bsTs = {}
                for p in pairs:
                    # block sums live in row 127; PSUM reads need 32-aligned
                    # partition base: copy rows 96:128, then scatter row 31
                    c32 = c32p.tile([32, NBLK, 2 * EH + 1], f16, tag="c32")
                    nc.scalar.copy(c32[:, :, 0:EH], pss[(p, 0)][96:128, :, :])
                    nc.vector.tensor_copy(
                        c32[:, :, EH:2 * EH], pss[(p, 1)][96:128, :, :])
                    nc.vector.tensor_copy(
                        c32[:, :, 2 * EH:2 * EH + 1].rearrange("p k o -> p (k o)"),
                        pss[(p, 2)][96:128, :])
                    bsT = bsp.tile([NBLK, 1, 2 * EH + 1], f16, tag="bs")
                    nc.sync.dma_start(
                        out=bsT[:], in_=c32[31:32, :, :])
                    bsTs[p] = bsT

                rms = {}
                for p in pairs:
                    rm = rmp.tile([NBLK, NBLK, 2 * EH + 1], f16, tag="rm")
                    nc.vector.tensor_tensor(
                        out=rm[:], in0=maskKED[:],
                        in1=bsTs[p][:].broadcast_to([NBLK, NBLK, 2 * EH + 1]),
                        op=mybir.AluOpType.mult,
                    )
                    rms[p] = rm

                for p in pairs:
                    nc.tensor.matmul(
                        pss[(p, 0)][:], lhsT=ones16[:], rhs=rms[p][:, :, 0:EH],
                        start=False, stop=True, skip_group_check=True,
                    )
                    nc.tensor.matmul(
                        pss[(p, 1)][:], lhsT=ones16[:],
                        rhs=rms[p][:, :, EH:2 * EH],
                        start=False, stop=True, skip_group_check=True,
                    )
                    nc.tensor.matmul(
                        pss[(p, 2)][:], lhsT=ones16[:],
                        rhs=rms[p][:, :, 2 * EH:2 * EH + 1].rearrange(
                            "a k o -> a (k o)"),
                        start=False, stop=True, skip_group_check=True,
                    )

                for p in pairs:
                    r = rp.tile([128, 1, NBLK], f16, tag="r")
                    nc.vector.reciprocal(
                        r[:].rearrange("p o k -> p (o k)"), pss[(p, 2)][:])
                    cw = cwp.tile([128, E, NBLK], f16, tag="cw")
                    # transposed drains: PSUM (k,e) -> SBUF e-major (e,k)
                    nc.scalar.copy(
                        cw[:, 0:EH, :].rearrange("p e k -> p k e"),
                        pss[(p, 0)][:])
                    nc.scalar.copy(
                        cw[:, EH:2 * EH, :].rearrange("p e k -> p k e"),
                        pss[(p, 1)][:])
                    ot = otp.tile([128, E, NBLK], f16, tag="ot")
                    nc.vector.tensor_tensor(
                        out=ot[:], in0=cw[:],
                        in1=r[:].broadcast_to([128, E, NBLK]),
                        op=mybir.AluOpType.mult,
                    )
                    nc.sync.dma_start(out=out[p], in_=ot[:])

    nc.compile()
    return nc


def _get_compiled():
    global _compiled
    if _compiled is None:
        _compiled = _build()
    return _compiled


def _consts():
    f16 = np.float16
    tri = np.triu(np.ones((128, 128), np.float32)).astype(f16)  # tri[c,p]=1 iff c<=p
    ones16 = np.ones((16, 128), f16)
    mk = (np.arange(NBLK)[:, None] < np.arange(NBLK)[None, :]).astype(np.float32)
    mke = np.broadcast_to(mk[:, :, None], (16, NBLK, 2 * EH + 1)).astype(f16)
    return {
        "tri_c": tri,
        "ones_c": ones16,
        "mke_c": np.ascontiguousarray(mke),
    }


def prep_inputs(keys: np.ndarray, values: np.ndarray, w_score: np.ndarray):
    """Host-side reshard: returns in_maps (list of 8 dicts)."""
    keys = np.asarray(keys, dtype=np.float32)
    values = np.asarray(values, dtype=np.float32)
    w = np.asarray(w_score, dtype=np.float32)

    # [B,S,H,E] -> [B,H,S,E] -> [B*H, NBLK, 128, E] -> [B*H, 128, NBLK, E]
    kt = keys.transpose(0, 2, 1, 3).reshape(B * H, NBLK, 128, E)
    kt = (kt * (-SCALE * w)).transpose(0, 2, 1, 3).astype(np.float16)

    v = values.transpose(0, 2, 1, 3).reshape(B * H, NBLK, 128, E)
    v = v.transpose(0, 2, 3, 1)  # [B*H, 128, E, NBLK]
    vgf = np.concatenate(
        [v, np.ones((B * H, 128, 1, NBLK), np.float32)], axis=2
    ).astype(np.float16)  # [B*H, 128, E+1, NBLK]

    consts = _consts()
    in_maps = []
    for c in range(NCORES):
        sl = slice(PAIRS * c, PAIRS * (c + 1))
        m = {
            "ktw": np.ascontiguousarray(kt[sl]),
            "vg": np.ascontiguousarray(vgf[sl]),
        }
        m.update(consts)
        in_maps.append(m)
    return in_maps


def assemble_output(results) -> np.ndarray:
    # results[c]["out"]: [PAIRS, 128, E, NBLK]; s = 128*k + partition
    arr = np.stack([np.asarray(r["out"]) for r in results])
    arr = arr.reshape(B * H, 128, E, NBLK).astype(np.float32)
    arr = arr.transpose(0, 3, 1, 2).reshape(B, H, L, E).transpose(0, 2, 1, 3)
    return np.ascontiguousarray(arr)


def kernel(queries=None, keys=None, values=None, w_score=None, b_score=None, attn_mask=None, **_):
    global LAST_RESULTS
    from concourse.bass_utils import run_bass_kernel_spmd

    nc = _get_compiled()
    in_maps = prep_inputs(keys, values, w_score)
    res = run_bass_kernel_spmd(nc, in_maps, core_ids=list(range(NCORES)), trace=TRACE)
    LAST_RESULTS = res
    return assemble_output(res.results)


# revision 18
# speedup vs baseline: 1.1686x; 1.1686x over previous
"""MinusAttention kernel for Trainium2 (8 NeuronCores, Bass/Tile).

Math: score[i,j] = (w.q_i - w.k_j + b) / sqrt(E) with causal mask.
Within a softmax row i the w.q_i and b terms cancel, so

    weights[i,j] = g_j / sum_{j'<=i} g_j',   g_j = exp(-w.k_j / sqrt(E))
    out[i,:]     = (sum_{j<=i} g_j V[j,:]) / (sum_{j<=i} g_j)

i.e. a causal cumulative weighted average of V -- O(S*E) per (b,h) --
and the output does not depend on queries at all.

v4: fp16 end-to-end (PE 1 cycle/col vs 4 for fp32, half the HBM bytes)
with k-major layouts everywhere so every engine/matmul access pattern
is contiguous (measured: transposed ACT writes cost 3.5x, permuted
matmul rhs costs 2x).  One merged block-sum scatter per pair, carry
mask multiply in the DVE packed-16bit 2x mode, final normalize on the
otherwise-idle GPSIMD engine, consts DMA'd from host, vg loads on the
gpsimd SW queue / kt+scatter+out on the SP HW ring.

Per pair (b,h), s = 128*k + p (p partition, k block 0..15):
  kt  [128,16,64] f16  (host-prescaled by -w/sqrt(E))
  vg  [128,16,65] f16  (col e=64 is ones)
  sk  = reduce_add_e(kt)            DVE -> [128,16] f16
  g   = exp(sk)                     ACT
  wg  = vg * g_bcast                DVE
  ps_c = triT @ wg[:,4c:4c+4,:]     PE f16, c=0..3 (260 cols, 1 bank)
  c32[:,4c:4c+4,:] = ps_c[96:128]   ACT (PSUM reads: 32-aligned base)
  bsT = scatter c32 row 31          1 DMA (block sums -> partitions)
  rm  = maskKED * bsT_bcast         DVE 2x (carry terms, k' < k)
  ps_c += ones16 @ rm[:,4c:4c+4,:]  PE (adds inter-block carries)
  cw[:,4c:4c+4,:] = ps_c            ACT drains
  r   = 1/cw[:,:,64]                DVE
  ot  = cw[:,:,0:64] * r_bcast      GPSIMD
  out DMA (f16; host upcasts)
"""

import numpy as np

B, L, S, H, E = 4, 2048, 2048, 8, 64
NCORES = 8
PAIRS = (B * H) // NCORES  # 4 (b,h) pairs per core
NBLK = S // 128  # 16
CHUNK = 4  # blocks per PSUM tile: 4*65*4B = 1040B < 2KB bank
NCHUNK = NBLK // CHUNK  # 4
GROUP = 2
SCALE = np.float32(1.0 / np.sqrt(np.float32(E)))

TRACE = False
LAST_RESULTS = None

_compiled = None


def _build():
    from concourse import bacc
    import concourse.mybir as mybir
    import concourse.tile as tile

    f16 = mybir.dt.float16
    f32 = mybir.dt.float32
    nc = bacc.Bacc("TRN2", target_bir_lowering=False, debug=False)

    ktw = nc.dram_tensor("ktw", [PAIRS, 128, NBLK, E], f16, kind="ExternalInput")
    vg = nc.dram_tensor("vg", [PAIRS, 128, NBLK, E + 1], f16, kind="ExternalInput")
    tri_c = nc.dram_tensor("tri_c", [128, 128], f16, kind="ExternalInput")
    ones_c = nc.dram_tensor("ones_c", [16, 128], f16, kind="ExternalInput")
    # maskKED[k', k, e] = 1 iff k' < k (bcast along e incl. the D col)
    mke_c = nc.dram_tensor("mke_c", [16, NBLK, E + 1], f16, kind="ExternalInput")
    out = nc.dram_tensor("out", [PAIRS, 128, NBLK, E], f16, kind="ExternalOutput")

    with tile.TileContext(nc) as tc:
        with (
            nc.allow_low_precision(reason="fp16 kernel; harness gate is 2e-2"),
            tc.tile_pool(name="const", bufs=1) as cpool,
            tc.tile_pool(name="ktp", bufs=GROUP + 1) as ktp,
            tc.tile_pool(name="vgp", bufs=GROUP + 1) as vgp,
            tc.tile_pool(name="skp", bufs=2 * GROUP) as skp,
            tc.tile_pool(name="gp", bufs=2 * GROUP) as gp,
            tc.tile_pool(name="wgp", bufs=GROUP + 1) as wgp,
            tc.tile_pool(name="c32p", bufs=2 * GROUP) as c32p,
            tc.tile_pool(name="bsp", bufs=2 * GROUP) as bsp,
            tc.tile_pool(name="rmp", bufs=2 * GROUP) as rmp,
            tc.tile_pool(name="rp", bufs=2 * GROUP) as rp,
            tc.tile_pool(name="cwp", bufs=GROUP + 1) as cwp,
            tc.tile_pool(name="otp", bufs=GROUP + 1) as otp,
            tc.tile_pool(name="ps", bufs=8, space="PSUM") as psp,
        ):
            tri = cpool.tile([128, 128], f16)
            nc.sync.dma_start(out=tri[:], in_=tri_c[:])
            ones16 = cpool.tile([16, 128], f16)
            nc.sync.dma_start(out=ones16[:], in_=ones_c[:])
            maskKED = cpool.tile([16, NBLK, E + 1], f16)
            nc.sync.dma_start(out=maskKED[:], in_=mke_c[:])

            for grp in range(PAIRS // GROUP):
                pairs = list(range(grp * GROUP, (grp + 1) * GROUP))

                kts, vgts = {}, {}
                for p in pairs:
                    kt = ktp.tile([128, NBLK, E], f16, tag="kt")
                    vgt = vgp.tile([128, NBLK, E + 1], f16, tag="vg")
                    nc.sync.dma_start(out=kt[:], in_=ktw[p])
                    nc.gpsimd.dma_start(out=vgt[:], in_=vg[p])
                    kts[p], vgts[p] = kt, vgt

                wgs = {}
                for p in pairs:
                    sk = skp.tile([128, NBLK], f16, tag="sk")
                    nc.vector.tensor_reduce(
                        sk[:], kts[p][:], mybir.AxisListType.X, mybir.AluOpType.add
                    )
                    g = gp.tile([128, NBLK], f16, tag="g")
                    nc.scalar.activation(
                        g[:], sk[:], mybir.ActivationFunctionType.Exp
                    )
                    wg = wgp.tile([128, NBLK, E + 1], f16, tag="wg")
                    nc.vector.tensor_tensor(
                        out=wg[:], in0=vgts[p][:],
                        in1=g[:].to_broadcast([128, NBLK, E + 1]),
                        op=mybir.AluOpType.mult,
                    )
                    wgs[p] = wg

                pss = {}
                for p in pairs:
                    for c in range(NCHUNK):
                        ps = psp.tile([128, CHUNK, E + 1], f32, tag="ps")
                        nc.tensor.matmul(
                            ps[:], lhsT=tri[:],
                            rhs=wgs[p][:, c * CHUNK:(c + 1) * CHUNK, :],
                            start=True, stop=False, skip_group_check=True,
                        )
                        pss[(p, c)] = ps

                bsTs = {}
                for p in pairs:
                    # block sums live in row 127; PSUM reads need 32-aligned
                    # partition base: copy rows 96:128, then scatter row 31
                    c32 = c32p.tile([32, NBLK, E + 1], f16, tag="c32")
                    for c in range(NCHUNK):
                        nc.scalar.copy(
                            c32[:, c * CHUNK:(c + 1) * CHUNK, :],
                            pss[(p, c)][96:128, :, :])
                    bsT = bsp.tile([NBLK, 1, E + 1], f16, tag="bs")
                    nc.sync.dma_start(out=bsT[:], in_=c32[31:32, :, :])
                    bsTs[p] = bsT

                rms = {}
                for p in pairs:
                    rm = rmp.tile([NBLK, NBLK, E + 1], f16, tag="rm")
                    nc.vector.tensor_tensor(
                        out=rm[:], in0=maskKED[:],
                        in1=bsTs[p][:].broadcast_to([NBLK, NBLK, E + 1]),
                        op=mybir.AluOpType.mult,
                    )
                    rms[p] = rm

                for p in pairs:
                    for c in range(NCHUNK):
                        nc.tensor.matmul(
                            pss[(p, c)][:], lhsT=ones16[:],
                            rhs=rms[p][:, c * CHUNK:(c + 1) * CHUNK, :],
                            start=False, stop=True, skip_group_check=True,
                        )

                for p in pairs:
                    cw = cwp.tile([128, NBLK, E + 1], f16, tag="cw")
                    for c in range(NCHUNK):
                        nc.scalar.copy(
                            cw[:, c * CHUNK:(c + 1) * CHUNK, :],
                            pss[(p, c)][:])
                    r = rp.tile([128, NBLK], f16, tag="r")
                    nc.vector.reciprocal(
                        r[:], cw[:, :, E:E + 1].rearrange("p k o -> p (k o)"))
                    ot = otp.tile([128, NBLK, E], f16, tag="ot")
                    nc.gpsimd.tensor_tensor(
                        out=ot[:], in0=cw[:, :, 0:E],
                        in1=r[:].to_broadcast([128, NBLK, E]),
                        op=mybir.AluOpType.mult,
                    )
                    nc.sync.dma_start(out=out[p], in_=ot[:])

    nc.compile()
    return nc


def _get_compiled():
    global _compiled
    if _compiled is None:
        _compiled = _build()
    return _compiled


def _consts():
    f16 = np.float16
    tri = np.triu(np.ones((128, 128), np.float32)).astype(f16)  # tri[c,p]=1 iff c<=p
    ones16 = np.ones((16, 128), f16)
    mk = (np.arange(NBLK)[:, None] < np.arange(NBLK)[None, :]).astype(np.float32)
    mke = np.broadcast_to(mk[:, :, None], (16, NBLK, E + 1)).astype(f16)
    return {
        "tri_c": tri,
        "ones_c": ones16,
        "mke_c": np.ascontiguousarray(mke),
    }


def prep_inputs(keys: np.ndarray, values: np.ndarray, w_score: np.ndarray):
    """Host-side reshard: returns in_maps (list of 8 dicts)."""
    keys = np.asarray(keys, dtype=np.float32)
    values = np.asarray(values, dtype=np.float32)
    w = np.asarray(w_score, dtype=np.float32)

    # [B,S,H,E] -> [B,H,S,E] -> [B*H, NBLK, 128, E] -> [B*H, 128, NBLK, E]
    kt = keys.transpose(0, 2, 1, 3).reshape(B * H, NBLK, 128, E)
    kt = (kt * (-SCALE * w)).transpose(0, 2, 1, 3).astype(np.float16)

    v = values.transpose(0, 2, 1, 3).reshape(B * H, NBLK, 128, E)
    v = v.transpose(0, 2, 1, 3)  # [B*H, 128, NBLK, E]
    vgf = np.concatenate(
        [v, np.ones((B * H, 128, NBLK, 1), np.float32)], axis=3
    ).astype(np.float16)  # [B*H, 128, NBLK, E+1]

    consts = _consts()
    in_maps = []
    for c in range(NCORES):
        sl = slice(PAIRS * c, PAIRS * (c + 1))
        m = {
            "ktw": np.ascontiguousarray(kt[sl]),
            "vg": np.ascontiguousarray(vgf[sl]),
        }
        m.update(consts)
        in_maps.append(m)
    return in_maps


def assemble_output(results) -> np.ndarray:
    # results[c]["out"]: [PAIRS, 128, NBLK, E]; s = 128*k + partition
    arr = np.stack([np.asarray(r["out"]) for r in results])
    arr = arr.reshape(B * H, 128, NBLK, E).astype(np.float32)
    arr = arr.transpose(0, 2, 1, 3).reshape(B, H, L, E).transpose(0, 2, 1, 3)
    return np.ascontiguousarray(arr)


def kernel(queries=None, keys=None, values=None, w_score=None, b_score=None, attn_mask=None, **_):
    global LAST_RESULTS
    from concourse.bass_utils import run_bass_kernel_spmd

    nc = _get_compiled()
    in_maps = prep_inputs(keys, values, w_score)
    res = run_bass_kernel_spmd(nc, in_maps, core_ids=list(range(NCORES)), trace=TRACE)
    LAST_RESULTS = res
    return assemble_output(res.results)
